# revision 1
# baseline (speedup 1.0000x reference)
"""Causal self-attention (B=4, T=2048, C=1024, H=16, D=64) on 8 trn2 NeuronCores.

Sharding: tensor-parallel over heads. Core g owns heads {2g, 2g+1}:
  - W_attn columns for those heads' q/k/v (128 cols each) -> per-core [1024, 384]
  - W_proj rows for those heads' channels -> per-core [128, 1024]
Each core computes a full [8192, 1024] partial of the output projection;
the host sums the 8 partials (the "all-reduce" of row-parallel W_proj).

Device layout notes:
  - x is passed as X^T [C, B*T] so every matmul contracts over the partition dim.
  - Attention uses the S^T = K @ Q^T formulation: S^T tiles are [k_tok, q_tok]
    so exp(S)*mask and the P^T @ V matmul need no on-chip transposes of P.
  - The softmax normalizer l[q] = sum_k P[k,q] comes from a ones column
    appended to V (stationary operand): one PSUM accumulation yields [y^T ; l].
  - Normalization multiplies y^T by broadcast(1/l) (K=1 matmul broadcast).
  - The b-loop is software-pipelined: QKV projection + V-transposes of batch
    b+1 are interleaved with attention q-tiles of batch b so the TensorE
    stays busy while ScalarE works through the exp stream.
"""

import numpy as np

B, T, C, H, D = 4, 2048, 1024, 16, 64
NCORES = 8
BT = B * T                      # 8192
HPC = H // NCORES               # 2 heads per core
CPC = HPC * D                   # 128 channels per core
NC_CHUNKS = C // 128            # 8 contraction chunks of X^T
QW = 512                        # q-tile width (moving dim)
KW = 128                        # k-tile width (S^T partition dim)

_CACHE = {}
LAST_RESULTS = None             # test harness reads exec_time_ns from here


def _build_bass():
    import concourse.bass as bass
    import concourse.mybir as mybir
    import concourse.tile as tile
    from concourse import bacc
    from concourse.masks import make_identity, make_upper_triangular

    f32 = mybir.dt.float32
    f32r = mybir.dt.float32r
    Exp = mybir.ActivationFunctionType.Exp

    nc = bacc.Bacc()
    xt = nc.dram_tensor("xt", [C, BT], f32r, kind="ExternalInput")
    wg = nc.dram_tensor("wg", [C, 3 * CPC], f32r, kind="ExternalInput")
    bg = nc.dram_tensor("bg", [3 * CPC], f32, kind="ExternalInput")
    wp = nc.dram_tensor("wp", [CPC, C], f32r, kind="ExternalInput")
    outp = nc.dram_tensor("outp", [BT, C], f32, kind="ExternalOutput")

    with tile.TileContext(nc) as tc:
        with (
            tc.tile_pool(name="const", bufs=1) as cpool,
            tc.tile_pool(name="sb", bufs=2) as sb,
            tc.tile_pool(name="ps", bufs=2, space="PSUM") as ps,
        ):
            # ---- constants ----
            # memset/affine_select cannot emit f32r directly; build in f32 and
            # round-cast via tensor_copy (the cast IS the required f32r round).
            scratch = cpool.tile([128, 128], f32, tag="scratch")
            make_identity(nc, scratch)
            identity = cpool.tile([128, 128], f32r, tag="ident")
            nc.vector.tensor_copy(identity, scratch)
            # mask[k, q] = 1.0 where q >= k else 0 (upper triangular incl diag);
            # only feeds DVE multiplies, so plain f32 is fine.
            mask = cpool.tile([128, 128], f32, tag="mask")
            make_upper_triangular(nc, mask, val=1.0, diag=True)
            # broadcast mask over the head axis (free-dim stride 0)
            mask2 = bass.AP(
                tensor=mask.tensor, offset=mask.offset,
                ap=[mask.ap[0], [0, 2], mask.ap[1]],
            )
            scratch2 = cpool.tile([128, 64], f32, tag="scratch2")
            nc.gpsimd.memset(scratch2, 1.0)
            ones_row = cpool.tile([128, 64], f32r, tag="ones")
            nc.vector.tensor_copy(ones_row, scratch2)

            # ---- weights ----
            wg_sb = []
            for ci in range(NC_CHUNKS):
                wgt = cpool.tile([128, 3 * CPC], f32r, tag=f"wg{ci}")
                nc.sync.dma_start(out=wgt, in_=wg[ci * 128:(ci + 1) * 128, :])
                wg_sb.append(wgt)
            wp_sb = cpool.tile([CPC, C], f32r, tag="wp")
            nc.sync.dma_start(out=wp_sb, in_=wp[:, :])
            bias_sb = []
            for grp in range(3):
                bt_ = cpool.tile([128, 1], f32, tag=f"bias{grp}")
                nc.sync.dma_start(
                    out=bt_,
                    in_=bg[grp * 128:(grp + 1) * 128].rearrange("(p o) -> p o", o=1),
                )
                bias_sb.append(bt_)

            qkv = {}       # b -> (qt_sb, kt_sb, vt_sb)
            vaug = {}      # b -> list of 16 [128, 130] tiles ([V_A|1|V_B|1])
            pending_proj = []   # deferred (b, qt, yt_sb) -> proj runs one qt later

            def qkv_toktile(b, tt):
                """QKV projection for tokens [b*T + tt*QW, +QW)."""
                tok0 = b * T
                if tt == 0:
                    qt_sb = sb.tile([128, T], f32r, tag="qt", name="qt_sb")
                    kt_sb = sb.tile([128, T], f32r, tag="kt", name="kt_sb")
                    vt_sb = sb.tile([128, T], f32r, tag="vt", name="vt_sb")
                    qkv[b] = (qt_sb, kt_sb, vt_sb)
                    vaug[b] = []
                dests = qkv[b]
                xts = []
                for ci in range(NC_CHUNKS):
                    xtile = sb.tile([128, QW], f32r, tag="xt", bufs=16, name="xtile")
                    nc.sync.dma_start(
                        out=xtile,
                        in_=xt[ci * 128:(ci + 1) * 128,
                               tok0 + tt * QW: tok0 + (tt + 1) * QW],
                    )
                    xts.append(xtile)
                for grp in range(3):
                    pqkv = ps.tile([128, QW], f32, tag="mm", name="pqkv")
                    for ci in range(NC_CHUNKS):
                        nc.tensor.matmul(
                            pqkv,
                            wg_sb[ci][:, grp * 128:(grp + 1) * 128],
                            xts[ci],
                            start=(ci == 0),
                            stop=(ci == NC_CHUNKS - 1),
                        )
                    nc.vector.tensor_scalar_add(
                        out=dests[grp][:, tt * QW:(tt + 1) * QW],
                        in0=pqkv,
                        scalar1=bias_sb[grp],
                    )

            def vtrans(b, kt0, nkt=4):
                """V^T -> [V_A | 1 | V_B | 1] tiles for k-tiles kt0..kt0+nkt."""
                vt_sb = qkv[b][2]
                for kt in range(kt0, kt0 + nkt):
                    ptr = ps.tile([128, 128], f32r, tag="mm",
                                  padded_shape=[128, 512], name="ptr")
                    nc.tensor.transpose(
                        ptr, vt_sb[:, kt * KW:(kt + 1) * KW], identity
                    )
                    va = sb.tile([128, 2, D + 1], f32r, tag="vaug", bufs=36,
                                 name="va")
                    nc.vector.tensor_copy(
                        va[:, :, 0:D],
                        ptr[:, 0:2 * D].rearrange("p (h x) -> p h x", x=D),
                    )
                    nc.vector.tensor_copy(va[:, :, D:D + 1], ones_row[:, 0:2])
                    vaug[b].append(va)

            def attention_qtile(b, qt):
                qt_sb, kt_sb, _ = qkv[b]
                tok0 = b * T
                y2 = ps.tile([D + 1, 2, QW], f32, tag="y", bufs=1, name="y2")
                nkt = (qt + 1) * (QW // KW)
                kdiag = qt * (QW // KW)      # first diagonal k-tile
                for kt in range(nkt):
                    diag = kt >= kdiag
                    qoff = (kt - kdiag) * KW if diag else 0
                    w = QW - qoff
                    qsl = slice(qt * QW + qoff, (qt + 1) * QW)
                    ksl = slice(kt * KW, (kt + 1) * KW)
                    st = ps.tile([128, 2, QW], f32, tag="st", name="st")
                    nc.tensor.matmul(
                        st[:, 0, 0:w], kt_sb[0:64, ksl], qt_sb[0:64, qsl]
                    )
                    nc.tensor.matmul(
                        st[:, 1, 0:w], kt_sb[64:128, ksl], qt_sb[64:128, qsl]
                    )
                    p = sb.tile([128, 2, QW], f32r, tag="p", bufs=4, name="p")
                    nc.scalar.activation(
                        p[:, :, 0:w], st[:, :, 0:w], Exp, scale=1.0 / np.sqrt(D)
                    )
                    if diag:
                        nc.vector.tensor_mul(p[:, 0, 0:KW], p[:, 0, 0:KW], mask)
                        nc.vector.tensor_mul(p[:, 1, 0:KW], p[:, 1, 0:KW], mask)
                    va = vaug[b][kt]
                    nc.tensor.matmul(
                        y2[:, 0, qoff:QW], va[:, 0, :], p[:, 0, 0:w],
                        start=(kt == 0), stop=(kt == nkt - 1),
                    )
                    nc.tensor.matmul(
                        y2[:, 1, qoff:QW], va[:, 1, :], p[:, 1, 0:w],
                        start=(kt == 0), stop=(kt == nkt - 1),
                    )

                # ---- normalize: y^T * broadcast(1/l) -> yt_sb [128, QW] ----
                # Stage y PSUM -> SBUF immediately (frees the y bank for the
                # next q-tile) and run normalization from SBUF off the PE path.
                ystage = sb.tile([128, 2, QW], f32, tag="ystage", name="ystage")
                nc.vector.tensor_copy(ystage[0:D + 1, :, :], y2[0:D + 1, :, :])
                # 1/l on one DVE lane costs ~6.5 cyc/elem; spread the 1024
                # l-values over 32 partitions with a 32x32 stream-transpose,
                # reciprocal there (32 elems/lane), and transpose back.
                lrow = ystage.rearrange("p h q -> p (h q)")
                lt = sb.tile([128, 2 * QW], f32, tag="lt", name="lt")
                nc.vector.transpose(lt[D:D + 32, :], lrow[D:D + 32, :])
                rt = sb.tile([128, 2 * QW], f32, tag="rt", name="rt")
                lt_v = lt[D:D + 32, :].rearrange("p (j c) -> p j c", c=32)
                rt_v = rt[D:D + 32, :].rearrange("p (j c) -> p j c", c=32)
                nc.vector.reciprocal(rt_v[:, :, 0:1], lt_v[:, :, 0:1])
                rcf = sb.tile([128, 2 * QW], f32, tag="rcf", name="rcf")
                nc.vector.transpose(rcf[D:D + 32, :], rt[D:D + 32, :])
                rcr = sb.tile([128, 2 * QW], f32r, tag="recipr", name="rcr")
                nc.vector.tensor_copy(rcr[D:D + 1, :], rcf[D:D + 1, :])
                bca = ps.tile([64, QW], f32, tag="mm", name="bca")
                bcb = ps.tile([64, QW], f32, tag="mm", name="bcb")
                nc.tensor.matmul(bca, ones_row[D:D + 1, 0:64], rcr[D:D + 1, 0:QW])
                nc.tensor.matmul(bcb, ones_row[D:D + 1, 0:64], rcr[D:D + 1, QW:2 * QW])
                yt_sb = sb.tile([128, QW], f32r, tag="yt", name="yt_sb")
                nc.vector.tensor_mul(yt_sb[0:64, :], ystage[0:D, 0, :], bca)
                nc.vector.tensor_mul(yt_sb[64:128, :], ystage[0:D, 1, :], bcb)

                pending_proj.append((b, qt, yt_sb))

            def proj_qtile(b, qt, yt_sb):
                tok0 = b * T
                for m in range(QW // 128):
                    osb = sb.tile([128, C], f32, tag="osb", bufs=3, name="osb")
                    for n in range(C // 512):
                        pp = ps.tile([128, 512], f32, tag="mm", name="pp")
                        nc.tensor.matmul(
                            pp, yt_sb[:, m * 128:(m + 1) * 128],
                            wp_sb[:, n * 512:(n + 1) * 512],
                        )
                        if n == 0:
                            nc.scalar.copy(osb[:, n * 512:(n + 1) * 512], pp)
                        else:
                            nc.vector.tensor_copy(osb[:, n * 512:(n + 1) * 512], pp)
                    row0 = tok0 + qt * QW + m * 128
                    nc.gpsimd.dma_start(out=outp[row0:row0 + 128, :], in_=osb)

            # ---- software-pipelined schedule ----
            for tt in range(T // QW):
                qkv_toktile(0, tt)
                vtrans(0, tt * 4)
            for b in range(B):
                for qt in range(T // QW):
                    deferred = pending_proj[:]
                    pending_proj.clear()
                    attention_qtile(b, qt)
                    for args in deferred:
                        proj_qtile(*args)
                    if b + 1 < B:
                        qkv_toktile(b + 1, qt)
                        vtrans(b + 1, qt * 4)
            for args in pending_proj:
                proj_qtile(*args)
            pending_proj.clear()

    nc.finalize()
    return nc


def _get_nc():
    if "nc" not in _CACHE:
        _CACHE["nc"] = _build_bass()
    return _CACHE["nc"]


def kernel(x, W_attn, b_attn, W_proj, b_proj):
    global LAST_RESULTS
    from concourse import bass_utils

    x = np.asarray(x, dtype=np.float32)
    W_attn = np.asarray(W_attn, dtype=np.float32)
    b_attn = np.asarray(b_attn, dtype=np.float32)
    W_proj = np.asarray(W_proj, dtype=np.float32)
    b_proj = np.asarray(b_proj, dtype=np.float32)

    xt_full = np.ascontiguousarray(x.reshape(BT, C).T)  # [C, B*T]

    in_maps = []
    for g in range(NCORES):
        cols = slice(g * CPC, (g + 1) * CPC)
        wg_g = np.ascontiguousarray(np.concatenate(
            [W_attn[:, cols], W_attn[:, C:][:, cols], W_attn[:, 2 * C:][:, cols]],
            axis=1,
        ))
        bg_g = np.ascontiguousarray(np.concatenate(
            [b_attn[cols], b_attn[C:][cols], b_attn[2 * C:][cols]]
        ))
        wp_g = np.ascontiguousarray(W_proj[cols, :])
        in_maps.append({"xt": xt_full, "wg": wg_g, "bg": bg_g, "wp": wp_g})

    nc = _get_nc()
    res = bass_utils.run_bass_kernel_spmd(nc, in_maps, core_ids=list(range(NCORES)))
    LAST_RESULTS = res

    acc = np.zeros((BT, C), dtype=np.float64)
    for r_ in res.results:
        acc += r_["outp"]
    acc += b_proj
    return acc.astype(np.float32).reshape(B, T, C)



# revision 6
# speedup vs baseline: 1.2579x; 1.2579x over previous
"""Causal self-attention (B=4, T=2048, C=1024, H=16, D=64) on 8 trn2 NeuronCores.

Sharding: (batch, head-half). Core g owns batch b=g//2 and head-half
hh=g%2 -> heads [8*hh, 8*hh+8) = 4 head-pairs, 512 channels. Each core
computes out_partial[2048, 1024] = y_half @ W_proj[rows of its half];
the host sums the two partials per batch (row-parallel W_proj) + b_proj.
vs. pure head-parallel sharding this cuts per-core DMA 4x (x in, out out)
at identical PE cycle count.

All matmul operands are bf16: the PE streams bf16 at 1 cyc/elem vs ~2 for
fp32, and fast-weight-load (disabled for fp32) hides LDWEIGHTS. PSUM
accumulation stays fp32; the softmax normalizer chain stays fp32.

Device structure per core (same S^T = K @ Q^T trick as before):
  - Attention S^T tiles are [k_tok, q_tok]: exp(S)*mask and P^T @ V need no
    on-chip transposes of P. The normalizer l[q] comes from a ones column
    appended to V; y^T * broadcast(1/l) via a K=1 matmul broadcast.
  - kt-loop is software-pipelined depth 1: S(kt+1) is issued before PV(kt)
    so the PE streams S while ScalarE works exp(kt).
  - QKV projection / V-transposes / output projection of neighboring tiles
    are interleaved between attention q-tiles to keep the PE dense.
"""

import numpy as np

B, T, C, H, D = 4, 2048, 1024, 16, 64
NCORES = 8
HPC = 8                  # heads per core
NHP = 4                  # head-pairs per core
CPC = HPC * D            # 512 channels per core
NC_CHUNKS = C // 128     # 8 contraction chunks of X^T
GC = 3 * CPC             # 1536 qkv-projection output cols per core
QW = 512                 # q-tile width (moving dim)
KW = 128                 # k-tile width (S^T partition dim)
NTT = T // QW            # 4 token tiles
SCALE = 1.0 / np.sqrt(D)

_CACHE = {}
LAST_RESULTS = None      # test harness reads exec_time_ns from here


def _build_bass():
    import concourse.bass as bass
    import concourse.mybir as mybir
    import concourse.tile as tile
    from concourse import bacc
    from concourse.masks import make_identity, make_upper_triangular

    f32 = mybir.dt.float32
    bf16 = mybir.dt.bfloat16
    Exp = mybir.ActivationFunctionType.Exp

    nc = bacc.Bacc()
    xt = nc.dram_tensor("xt", [C, T], bf16, kind="ExternalInput")
    wg = nc.dram_tensor("wg", [C, GC], bf16, kind="ExternalInput")
    bg = nc.dram_tensor("bg", [GC], f32, kind="ExternalInput")
    wp = nc.dram_tensor("wp", [CPC, C], bf16, kind="ExternalInput")
    outp = nc.dram_tensor("outp", [T, C], f32, kind="ExternalOutput")

    with tile.TileContext(nc) as tc:
        with (
            tc.tile_pool(name="const", bufs=1) as cpool,
            tc.tile_pool(name="sb", bufs=2) as sb,
            tc.tile_pool(name="ps", bufs=2, space="PSUM") as ps,
        ):
            # ---- constants (built f32 via gpsimd, cast to bf16) ----
            scratch = cpool.tile([128, 128], f32, tag="scratch")
            make_identity(nc, scratch)
            identity = cpool.tile([128, 128], bf16, tag="ident")
            nc.vector.tensor_copy(identity, scratch)
            # mask[k, q] = 1.0 where q >= k else 0 (upper triangular incl diag)
            scratch2 = cpool.tile([128, 128], f32, tag="scratch2")
            make_upper_triangular(nc, scratch2, val=1.0, diag=True)
            mask = cpool.tile([128, 128], bf16, tag="mask")
            nc.vector.tensor_copy(mask, scratch2)
            # broadcast mask over the head axis (free-dim stride 0)
            mask2 = bass.AP(
                tensor=mask.tensor, offset=mask.offset,
                ap=[mask.ap[0], [0, 2], mask.ap[1]],
            )
            scratch3 = cpool.tile([128, 64], f32, tag="scratch3")
            nc.gpsimd.memset(scratch3, 1.0)
            ones_row = cpool.tile([128, 64], bf16, tag="ones")
            nc.vector.tensor_copy(ones_row, scratch3)

            # ---- weights ----
            wg_sb = []
            for ci in range(NC_CHUNKS):
                t_ = cpool.tile([128, GC], bf16, tag=f"wg{ci}")
                nc.sync.dma_start(out=t_, in_=wg[ci * 128:(ci + 1) * 128, :])
                wg_sb.append(t_)
            wp_sb = []
            for hp in range(NHP):
                t_ = cpool.tile([128, C], bf16, tag=f"wp{hp}")
                nc.sync.dma_start(out=t_, in_=wp[hp * 128:(hp + 1) * 128, :])
                wp_sb.append(t_)
            # bias chunk o = grp*4 + hp lives in bias_sb[:, o]
            bias_sb = cpool.tile([128, 12], f32, tag="bias")
            nc.sync.dma_start(out=bias_sb, in_=bg.rearrange("(o p) -> p o", p=128))

            # ---- per-core persistent state ----
            # qkv_sb[grp][hp]: [128, T] bf16, rows 0:64 head 2hp, 64:128 head 2hp+1
            qkv_sb = [
                [cpool.tile([128, T], bf16, tag=f"{nm}{hp}", name=f"{nm}{hp}")
                 for hp in range(NHP)]
                for nm in ("q", "k", "v")
            ]
            vaug = [[None] * (T // KW) for _ in range(NHP)]

            def qkv_toktile(tt):
                """QKV projection for tokens [tt*QW, (tt+1)*QW)."""
                sl = slice(tt * QW, (tt + 1) * QW)
                xts = []
                for ci in range(NC_CHUNKS):
                    xtile = sb.tile([128, QW], bf16, tag="xt", bufs=16, name="xtile")
                    nc.sync.dma_start(out=xtile, in_=xt[ci * 128:(ci + 1) * 128, sl])
                    xts.append(xtile)
                for o in range(12):
                    grp, hp = divmod(o, NHP)
                    pq = ps.tile([128, QW], f32, tag="mm", name="pq")
                    for ci in range(NC_CHUNKS):
                        nc.tensor.matmul(
                            pq, wg_sb[ci][:, o * 128:(o + 1) * 128], xts[ci],
                            start=(ci == 0), stop=(ci == NC_CHUNKS - 1),
                        )
                    # bias-add doubles as the PSUM->SBUF+cast copy
                    nc.vector.tensor_scalar_add(
                        out=qkv_sb[grp][hp][:, sl], in0=pq,
                        scalar1=bias_sb[:, o:o + 1],
                    )

            def vtrans_toktile(tt):
                """V^T -> [V_A | 1 | V_B | 1] tiles for this token tile."""
                for hp in range(NHP):
                    vt = qkv_sb[2][hp]
                    for kt in range(tt * 4, tt * 4 + 4):
                        ptr = ps.tile([128, 128], bf16, tag="mm",
                                      padded_shape=[128, 1024], name="ptr")
                        nc.tensor.transpose(
                            ptr, vt[:, kt * KW:(kt + 1) * KW], identity
                        )
                        va = cpool.tile([128, 2, D + 1], bf16, tag=f"va{hp}_{kt}",
                                        name="va")
                        nc.vector.tensor_copy(
                            va[:, :, 0:D],
                            ptr[:, 0:2 * D].rearrange("p (h x) -> p h x", x=D),
                        )
                        nc.vector.tensor_copy(va[:, :, D:D + 1], ones_row[:, 0:2])
                        vaug[hp][kt] = va

            def attention_qtile(qt, hp, yts):
                qt_sb, kt_sb = qkv_sb[0][hp], qkv_sb[1][hp]
                y2 = ps.tile([D + 1, 2, QW], f32, tag="y", bufs=1, name="y2")
                nkt = (qt + 1) * (QW // KW)
                kdiag = qt * (QW // KW)      # first diagonal k-tile

                def s_mm(kt):
                    diag = kt >= kdiag
                    qoff = (kt - kdiag) * KW if diag else 0
                    w = QW - qoff
                    qsl = slice(qt * QW + qoff, (qt + 1) * QW)
                    ksl = slice(kt * KW, (kt + 1) * KW)
                    st = ps.tile([128, 2, QW], f32, tag="st", name="st")
                    nc.tensor.matmul(
                        st[:, 0, 0:w], kt_sb[0:64, ksl], qt_sb[0:64, qsl]
                    )
                    nc.tensor.matmul(
                        st[:, 1, 0:w], kt_sb[64:128, ksl], qt_sb[64:128, qsl]
                    )
                    p = sb.tile([128, 2, QW], bf16, tag="p", bufs=4, name="p")
                    nc.scalar.activation(
                        p[:, :, 0:w], st[:, :, 0:w], Exp, scale=SCALE
                    )
                    if diag:
                        nc.vector.tensor_mul(p[:, :, 0:KW], p[:, :, 0:KW], mask2)
                    return (p, qoff, w, kt)

                def pv_mm(t):
                    p, qoff, w, kt = t
                    va = vaug[hp][kt]
                    nc.tensor.matmul(
                        y2[:, 0, qoff:QW], va[:, 0, :], p[:, 0, 0:w],
                        start=(kt == 0), stop=(kt == nkt - 1),
                    )
                    nc.tensor.matmul(
                        y2[:, 1, qoff:QW], va[:, 1, :], p[:, 1, 0:w],
                        start=(kt == 0), stop=(kt == nkt - 1),
                    )

                # depth-1 pipeline: S(kt+1) issues before PV(kt) so the PE
                # streams S while ScalarE runs exp(kt)
                prev = s_mm(0)
                for kt in range(1, nkt):
                    cur = s_mm(kt)
                    pv_mm(prev)
                    prev = cur
                pv_mm(prev)

                # ---- normalize: y^T * broadcast(1/l) -> yt_sb [128, QW] ----
                ystage = sb.tile([128, 2, QW], f32, tag="ystage", name="ystage")
                nc.vector.tensor_copy(ystage[0:D + 1, :, :], y2[0:D + 1, :, :])
                # 1/l via 32x32 stream-transpose so 1024 l-values use 32 lanes
                lrow = ystage.rearrange("p h q -> p (h q)")
                lt = sb.tile([128, 2 * QW], f32, tag="lt", name="lt")
                nc.vector.transpose(lt[D:D + 32, :], lrow[D:D + 32, :])
                rt = sb.tile([128, 2 * QW], f32, tag="rt", name="rt")
                lt_v = lt[D:D + 32, :].rearrange("p (j c) -> p j c", c=32)
                rt_v = rt[D:D + 32, :].rearrange("p (j c) -> p j c", c=32)
                nc.vector.reciprocal(rt_v[:, :, 0:1], lt_v[:, :, 0:1])
                rcf = sb.tile([128, 2 * QW], f32, tag="rcf", name="rcf")
                nc.vector.transpose(rcf[D:D + 32, :], rt[D:D + 32, :])
                rcr = sb.tile([128, 2 * QW], bf16, tag="rcr", name="rcr")
                nc.vector.tensor_copy(rcr[D:D + 1, :], rcf[D:D + 1, :])
                bca = ps.tile([64, QW], f32, tag="mm", name="bca")
                bcb = ps.tile([64, QW], f32, tag="mm", name="bcb")
                nc.tensor.matmul(bca, ones_row[D:D + 1, 0:64], rcr[D:D + 1, 0:QW])
                nc.tensor.matmul(bcb, ones_row[D:D + 1, 0:64], rcr[D:D + 1, QW:2 * QW])
                yt_sb = sb.tile([128, QW], bf16, tag="yt", bufs=8, name="yt_sb")
                nc.vector.tensor_mul(yt_sb[0:64, :], ystage[0:D, 0, :], bca)
                nc.vector.tensor_mul(yt_sb[64:128, :], ystage[0:D, 1, :], bcb)
                yts[hp] = yt_sb

            def proj_qtile(qt, yts):
                for m in range(QW // 128):
                    osb = sb.tile([128, C], f32, tag="osb", bufs=3, name="osb")
                    for n in range(2):
                        pp = ps.tile([128, 512], f32, tag="mm", name="pp")
                        for hp in range(NHP):
                            nc.tensor.matmul(
                                pp, yts[hp][:, m * 128:(m + 1) * 128],
                                wp_sb[hp][:, n * 512:(n + 1) * 512],
                                start=(hp == 0), stop=(hp == NHP - 1),
                            )
                        if n == 0:
                            nc.scalar.copy(osb[:, 0:512], pp)
                        else:
                            nc.vector.tensor_copy(osb[:, 512:1024], pp)
                    row0 = qt * QW + m * 128
                    nc.gpsimd.dma_start(out=outp[row0:row0 + 128, :], in_=osb)

            # ---- software-pipelined schedule ----
            qkv_toktile(0)
            vtrans_toktile(0)
            pending = []
            for qt in range(NTT):
                yts = {}
                for hp in range(NHP):
                    attention_qtile(qt, hp, yts)
                    if hp == 0 and qt + 1 < NTT:
                        qkv_toktile(qt + 1)
                    if hp == 1 and qt + 1 < NTT:
                        vtrans_toktile(qt + 1)
                    if hp == 2 and pending:
                        proj_qtile(*pending.pop())
                pending.append((qt, yts))
            proj_qtile(*pending.pop())

    nc.finalize()
    return nc


def _get_nc():
    if "nc" not in _CACHE:
        _CACHE["nc"] = _build_bass()
    return _CACHE["nc"]


def kernel(x, W_attn, b_attn, W_proj, b_proj):
    global LAST_RESULTS
    from concourse import bass_utils
    import ml_dtypes

    bf = ml_dtypes.bfloat16
    x = np.asarray(x, dtype=np.float32)
    W_attn = np.asarray(W_attn, dtype=np.float32)
    b_attn = np.asarray(b_attn, dtype=np.float32)
    W_proj = np.asarray(W_proj, dtype=np.float32)
    b_proj = np.asarray(b_proj, dtype=np.float32)

    in_maps = []
    for g in range(NCORES):
        b, hh = divmod(g, 2)
        cols = slice(hh * CPC, (hh + 1) * CPC)
        wg_g = np.ascontiguousarray(np.concatenate(
            [W_attn[:, cols], W_attn[:, C:][:, cols], W_attn[:, 2 * C:][:, cols]],
            axis=1,
        ).astype(bf))
        bg_g = np.ascontiguousarray(np.concatenate(
            [b_attn[cols], b_attn[C:][cols], b_attn[2 * C:][cols]]
        ))
        wp_g = np.ascontiguousarray(W_proj[cols, :].astype(bf))
        xt_g = np.ascontiguousarray(x[b].T.astype(bf))
        in_maps.append({"xt": xt_g, "wg": wg_g, "bg": bg_g, "wp": wp_g})

    nc = _get_nc()
    res = bass_utils.run_bass_kernel_spmd(nc, in_maps, core_ids=list(range(NCORES)))
    LAST_RESULTS = res

    out = np.empty((B, T, C), dtype=np.float32)
    for b in range(B):
        acc = res.results[2 * b]["outp"].astype(np.float64)
        acc += res.results[2 * b + 1]["outp"]
        acc += b_proj
        out[b] = acc.astype(np.float32)
    return out
